# revision 34
# baseline (speedup 1.0000x reference)
"""Trainium2 Bass kernel for nn_AttentionCell (Bahdanau attention + LSTM step).

Distribution over 8 cores: attention data-parallel over batch (4 per core);
LSTM tensor-parallel over U (128 cols/core, gate-interleaved). One AllGather
moves the normalized per-batch context rows between the two phases.

Key optimizations vs the original baseline (sim: 157us -> 94us/exec):
  - ONE AllGather instead of two (ctx rows are normalized per-core before the
    gather; the collective model charges ~15us fixed per collective). Payload
    is bf16.
  - Engine rebalance: e+q add and tanh*va mult on Pool (gpsimd), reduce on
    DVE, tanh on ACT. (TensorTensorReduce fusion crashes the device - the
    worker hangs - and float32r is rejected by the BIR verifier, both
    re-confirmed on silicon this session; do NOT re-enable.)
  - bf16 weights for q (Wa, h^T) and the LSTM z-path (kernel/rec_kernel/x):
    1 cy/row matmuls vs 4 for f32, and half the DMA bytes. Wa_b is folded
    host-side into a 9th contraction chunk of Wa. rel_err ~2.3e-3 (tol 2e-2).
  - DMA stream order tuned: enc/speech per-batch interleave on the SP queue,
    Wa halves early on the ACT queue (q matmuls start after the first half),
    last speech tile split so its ctx matmuls overlap the second half, LSTM
    weights ride after the stream (pool-tag trick pins them there - the Tile
    scheduler hoists dependency-free DMAs).
  - Per-chunk exp + chunk-major ctx matmuls; speech tiles copied to bf16 on
    ACT after landing so the ctx matmuls run at 1 cy/row; the softmax
    denominator accumulates per-chunk into a single PSUM cell; the last
    batch's enc/speech DMAs are split in halves and its mults alternate
    DVE/Pool — together these hide the last-batch tail under the stream.
  - LSTM x/h partial matmuls + PE-warming filler during the collective
    window (an idle PE drops from 2.4 to 1.2 GHz pstate; the filler keeps
    the tail z matmuls at full speed). A dummy sigmoid preloads the gate
    activation table early.
  - Small constants packed into one inline tensor -> one DMA (the cost model
    charges ~1.5-3us fixed per DMA instruction regardless of size).
"""

import numpy as np

_B, _T, _U, _E, _DIN = 32, 512, 1024, 1024, 256
_R = 8
_BL = _B // _R  # 4 batches per core
_UL = _U // _R  # 128 U-cols per core
_TC = _T // 128  # 4 T-chunks per batch
_KX = (_DIN + _E) // 128  # 10 x-chunks
_KH = _U // 128  # 8 h-chunks
_KQ = _KH + 1  # q-contraction chunks incl. the Wa_b fold row

_CACHE = {}
_F32R = False  # f32r rejected by the BIR verifier on this toolchain (k=1 mms)
_USE_TTR = False  # TensorTensorReduce crashes the device (worker hang) on HW
_POOL_ADD = True  # Pool-engine adds validated on HW


def _build(mode="full", f32r=None, use_ttr=None, pool_add=None):
    import concourse.bacc as bacc
    from concourse import mybir
    from concourse.tile import TileContext

    if f32r is None:
        f32r = _F32R
    if use_ttr is None:
        use_ttr = _USE_TTR
    if pool_add is None:
        pool_add = _POOL_ADD
    f32 = mybir.dt.float32
    nc = bacc.Bacc("TRN2", target_bir_lowering=False, debug=False, num_devices=_R)

    # ---- per-core I/O (shards prepared host-side in kernel()) ----
    xinT = nc.declare_dram_parameter("xinT", [_DIN, _B], mybir.dt.bfloat16, isOutput=False)
    hT = nc.declare_dram_parameter("hT", [_U, _B], mybir.dt.bfloat16, isOutput=False)
    hTc = nc.declare_dram_parameter("hTc", [_KQ * 128, _BL], mybir.dt.bfloat16, isOutput=False)
    c_sh = nc.declare_dram_parameter("c_sh", [_B, _UL], f32, isOutput=False)
    enc = nc.declare_dram_parameter("enc", [_BL, _T, _U], mybir.dt.bfloat16, isOutput=False)
    spe = nc.declare_dram_parameter("spe", [_BL, _T, _E], mybir.dt.bfloat16, isOutput=False)
    wa = nc.declare_dram_parameter("wa", [_KQ * 128, _U], mybir.dt.bfloat16, isOutput=False)
    va = nc.declare_dram_parameter("va", [1, _U], f32, isOutput=False)
    ker = nc.declare_dram_parameter("ker", [_DIN + _E, 4 * _UL], mybir.dt.bfloat16, isOutput=False)
    rec = nc.declare_dram_parameter("rec", [_U, 4 * _UL], mybir.dt.bfloat16, isOutput=False)
    bia = nc.declare_dram_parameter("bia", [1, 4 * _UL], f32, isOutput=False)
    out = nc.declare_dram_parameter("out", [2, _B, _UL], f32, isOutput=True)

    # ---- packed constants: one inline tensor, one DMA ----
    # cols 0:32    ident32 (rows 0..31)
    # col  32      ones column (all 128 rows)
    # cols 33:545  sel4: sel[b, b*128+j] = 1 (rows 0..3)
    # cols 545:549 g16: g[k, k//TC] = 1 (rows 0..15)
    # cols 549:677 ones row (row 0)
    W = 680
    cp = np.zeros((128, W), np.float32)
    cp[0:32, 0:32] = np.eye(32, dtype=np.float32)
    cp[:, 32] = 1.0
    for b in range(_BL):
        cp[b, 33 + b * 128 : 33 + (b + 1) * 128] = 1.0
    for k in range(_BL * _TC):
        cp[k, 545 + k // _TC] = 1.0
    cp[0, 549:677] = 1.0
    cp_d = nc.inline_tensor(cp, name="cpack")

    # collective bounce buffers (one AllGather: normalized ctx)
    cc_in = nc.dram_tensor("cc_in", [_BL, _E], mybir.dt.bfloat16)
    cc_out = nc.dram_tensor("cc_out", [_B, _E], mybir.dt.bfloat16, addr_space="Shared")

    AF = mybir.ActivationFunctionType
    ALU = mybir.AluOpType

    def _r(ap):
        return ap.bitcast(mybir.dt.float32r) if f32r is True else ap

    def _rd(ap):
        # deep-contraction matmuls only (k>=4): f32r in "deep" or True mode
        return ap.bitcast(mybir.dt.float32r) if f32r else ap

    with TileContext(nc) as tc:
        with (
            tc.tile_pool(name="const", bufs=1) as constp,
            tc.tile_pool(name="weights", bufs=1) as wp,
            tc.tile_pool(name="enc_p", bufs=3) as encp,
            tc.tile_pool(name="spe_p", bufs=3) as spep,
            tc.tile_pool(name="add_p", bufs=2) as addp,
            tc.tile_pool(name="scr_p", bufs=2) as scrp,
            tc.tile_pool(name="small", bufs=1) as smallp,
            tc.tile_pool(name="psqb", bufs=1, space="PSUM") as psqb,
            tc.tile_pool(name="psmm", bufs=1, space="PSUM") as psmm,
            tc.tile_pool(name="psz", bufs=1, space="PSUM") as psz,
            tc.tile_pool(name="pstp", bufs=1, space="PSUM") as pstp,
        ):
            # ---------- constants + small inputs ----------
            cpk = constp.tile([128, W], f32)
            nc.scalar.dma_start(cpk[:], cp_d[:])
            ident_t = cpk[0:32, 0:32]
            onescol_t = cpk[0:128, 32:33]
            sel_t = cpk[0:_BL, 33 : 33 + _BL * 128]
            g_t = cpk[0 : _BL * _TC, 545 : 545 + _BL]
            ones_t = cpk[0:1, 549 : 549 + 128]

            # hTc + wa gate the query matmul: load them first; wa in halves
            # so the first-half q matmuls overlap the second half's transfer
            hTc_t = wp.tile([128, _KQ, _BL], mybir.dt.bfloat16)
            nc.scalar.dma_start(
                hTc_t[:], hTc.ap().rearrange("(n p) b -> p n b", p=128)
            )
            wa_t = wp.tile([128, _KQ, _U], mybir.dt.bfloat16)
            for hh in range(2):
                nc.scalar.dma_start(
                    wa_t[:, :, hh * 512 : (hh + 1) * 512],
                    wa.ap()[:, hh * 512 : (hh + 1) * 512].rearrange(
                        "(n p) u -> p n u", p=128
                    ),
                )
            va_row = constp.tile([1, _U], f32)
            nc.scalar.dma_start(va_row[:], va[:])

            # LSTM x/misc tiles; ker/rec are allocated post-loop from the
            # speech pool so their DMAs genuinely depend on late buffers
            # (keeps the scheduler from hoisting them into the stream).
            xt = wp.tile([128, _KX + _KH, _B], mybir.dt.bfloat16)
            bia_t = smallp.tile([1, 4 * _UL], f32)
            c_t = smallp.tile([_B, _UL], f32)

            # PE warm-up: tiny k=1 matmuls on constants ramp the PE pstate
            # (0.65 -> 2.4 GHz needs ~3us continuous busy) before the
            # latency-critical q matmuls arrive
            warm0 = pstp.tile([1, 128], f32, tag="den")
            for _w in range(6):
                nc.tensor.matmul(
                    warm0[:],
                    ones_t[0:1, 0:1],
                    cpk[0:1, 0:128],
                    start=True,
                    stop=True,
                )

            # va broadcast to 128 partitions: ones_row^T-style k=1 matmul
            va_ps = psqb.tile([128, _U], f32, tag="qb")
            for hh in range(2):
                nc.tensor.matmul(
                    va_ps[:, hh * 512 : (hh + 1) * 512],
                    _r(ones_t),
                    _r(va_row[:, hh * 512 : (hh + 1) * 512]),
                    start=True,
                    stop=True,
                )
            va_bc = constp.tile([128, _U], mybir.dt.bfloat16)
            nc.vector.tensor_copy(va_bc[:], va_ps[:])
            sel_b = constp.tile([_BL, _BL * 128], mybir.dt.bfloat16)
            nc.vector.tensor_copy(sel_b[:], sel_t)
            onescol_b = constp.tile([128, 1], mybir.dt.bfloat16)
            nc.vector.tensor_copy(onescol_b[:], onescol_t)

            # ---------- query: q = h_core @ Wa_w + Wa_b -> q_sb [4, U] ----------
            q_ps = psmm.tile([_BL, _U], f32, tag="mm")
            for hh in range(2):
                for n in range(_KQ):
                    nc.tensor.matmul(
                        q_ps[:, hh * 512 : (hh + 1) * 512],
                        hTc_t[:, n, :],
                        wa_t[:, n, hh * 512 : (hh + 1) * 512],
                        start=(n == 0),
                        stop=(n == _KQ - 1),
                    )

            q_sb = smallp.tile([_BL, _U], mybir.dt.bfloat16)
            nc.vector.tensor_copy(q_sb[:], q_ps[:])
            # dummy sigmoid: force the gate act-table load early (off the tail)
            sig_warm = smallp.tile([1, 4], f32)
            nc.scalar.activation(sig_warm[:], q_sb[0:1, 0:4], AF.Sigmoid)

            # ---------- attention over this core's 4 batches ----------
            score = smallp.tile([128, _BL * _TC], f32)
            exp_s = smallp.tile([128, _BL * _TC], mybir.dt.bfloat16)

            for b in range(_BL):
                # q[b] broadcast to [128, U] PSUM
                qb_ps = psqb.tile([128, _U], f32, tag="qb")
                for hh in range(2):
                    nc.tensor.matmul(
                        qb_ps[:, hh * 512 : (hh + 1) * 512],
                        sel_b[:, b * 128 : (b + 1) * 128],
                        q_sb[:, hh * 512 : (hh + 1) * 512],
                        start=True,
                        stop=True,
                    )
                # Pool can't read PSUM (BIR verifier) -> stage qb in SBUF
                # (copy on DVE: the ACT queue is busy issuing DMAs early)
                qb_sb = scrp.tile([128, _U], mybir.dt.bfloat16, tag="qbs")
                nc.vector.tensor_copy(qb_sb[:], qb_ps[:])
                e_bt = encp.tile([128, _TC, _U], mybir.dt.bfloat16)
                if b == _BL - 1:
                    # last batch: halves, so its add/tanh/mult/reduce chain
                    # starts ~3us earlier and finishes before the speech gate
                    for ehalf in range(2):
                        nc.sync.dma_start(
                            e_bt[:, ehalf * 2 : (ehalf + 1) * 2, :],
                            enc[b][ehalf * 256 : (ehalf + 1) * 256].rearrange(
                                "(c p) u -> p c u", p=128
                            ),
                        )
                else:
                    nc.sync.dma_start(
                        e_bt[:], enc[b].rearrange("(c p) u -> p c u", p=128)
                    )
                for cch in range(_TC):
                    # add on Pool (or DVE); tanh on ACT; mult+reduce fused
                    # on DVE via TTR (or split mult/reduce)
                    a_t = addp.tile([128, _U], mybir.dt.bfloat16)
                    add_eng = nc.gpsimd if pool_add else nc.vector
                    add_eng.tensor_tensor(
                        out=a_t[:], in0=e_bt[:, cch, :], in1=qb_sb[:], op=ALU.add
                    )
                    nc.scalar.activation(e_bt[:, cch, :], a_t[:], AF.Tanh)
                    scr = scrp.tile([128, _U], mybir.dt.bfloat16)
                    if use_ttr:
                        nc.vector.tensor_tensor_reduce(
                            out=scr[:],
                            in0=e_bt[:, cch, :],
                            in1=va_bc[:],
                            scale=1.0,
                            scalar=0.0,
                            op0=ALU.mult,
                            op1=ALU.add,
                            accum_out=score[:, b * _TC + cch : b * _TC + cch + 1],
                        )
                    else:
                        mul_eng = nc.gpsimd if pool_add else nc.vector
                        if b == _BL - 1 and cch % 2 == 0:
                            mul_eng = nc.vector
                        mul_eng.tensor_tensor(
                            out=scr[:], in0=e_bt[:, cch, :], in1=va_bc[:],
                            op=ALU.mult,
                        )
                        nc.vector.tensor_reduce(
                            out=score[:, b * _TC + cch : b * _TC + cch + 1],
                            in_=scr[:],
                            axis=mybir.AxisListType.X,
                            op=ALU.add,
                        )
                s_bf = spep.tile([128, _TC, _E], mybir.dt.bfloat16, tag="sst")
                if b == _BL - 1:
                    for chalf in range(2):
                        nc.sync.dma_start(
                            s_bf[:, chalf * 2 : (chalf + 1) * 2, :],
                            spe[b][chalf * 256 : (chalf + 1) * 256].rearrange(
                                "(c p) u -> p c u", p=128
                            ),
                        )
                else:
                    nc.sync.dma_start(
                        s_bf[:], spe[b].rearrange("(c p) u -> p c u", p=128)
                    )
                # per-chunk exp + chunk-major context matmuls: chunk c's
                # matmuls start as soon as its reduce lands (b3 tail shrinks);
                # the denominator accumulates per-chunk into one PSUM cell
                ctxr_ps = psmm.tile([1, _E], f32, tag="mm")
                d1_ps = pstp.tile([1, 1], f32, tag="den")
                for cch in range(_TC):
                    col = b * _TC + cch
                    nc.scalar.activation(
                        exp_s[:, col : col + 1], score[:, col : col + 1], AF.Exp
                    )
                    nc.tensor.matmul(
                        d1_ps[:],
                        exp_s[:, col : col + 1],
                        onescol_b[:],
                        start=(cch == 0),
                        stop=(cch == _TC - 1),
                    )
                    for hh in range(2):
                        nc.tensor.matmul(
                            ctxr_ps[0:1, hh * 512 : (hh + 1) * 512],
                            exp_s[:, col : col + 1],
                            s_bf[:, cch, hh * 512 : (hh + 1) * 512],
                            start=(cch == 0),
                            stop=(cch == _TC - 1),
                        )
                recip1 = smallp.tile([1, 1], f32, tag=f"re_{b}")
                nc.vector.reciprocal(recip1[:], d1_ps[:])
                ctxr_sb = addp.tile([1, _E], mybir.dt.bfloat16, tag="ctxr")
                nc.vector.tensor_scalar_mul(ctxr_sb[:], ctxr_ps[:], recip1[:])
                nc.sync.dma_start(cc_in[b : b + 1, :], ctxr_sb[:])


            # deferred LSTM loads on the scalar queue (DMA device is free
            # now; ACT is done with tanh/exp); z x/h partials run on PE
            # during the collective window.
            nc.sync.dma_start(
                xt[:, 0:2, :], xinT.ap().rearrange("(n p) b -> p n b", p=128)
            )
            nc.sync.dma_start(
                xt[:, _KX : _KX + _KH, :],
                hT.ap().rearrange("(n p) b -> p n b", p=128),
            )
            ker_t = spep.tile([128, _KX, 4 * _UL], mybir.dt.bfloat16, tag="sst")
            nc.sync.dma_start(
                ker_t[:], ker.ap().rearrange("(n p) c -> p n c", p=128)
            )
            rec_t = spep.tile([128, _KH, 4 * _UL], mybir.dt.bfloat16, tag="sst")
            nc.sync.dma_start(
                rec_t[:], rec.ap().rearrange("(n p) c -> p n c", p=128)
            )
            nc.sync.dma_start(bia_t[:], bia[:])
            nc.sync.dma_start(c_t[:], c_sh[:])

            # ---------- AllGather normalized ctx ----------
            if mode == "full":
                nc.gpsimd.collective_compute(
                    "AllGather",
                    ALU.bypass,
                    replica_groups=[list(range(_R))],
                    ins=[cc_in.ap().opt()],
                    outs=[cc_out.ap().opt()],
                )
            else:  # debug: fill cc_out with own rows (wrong data, same dataflow)
                for rr in range(_R):
                    nc.sync.dma_start(
                        cc_out[rr * _BL : (rr + 1) * _BL, :], cc_in[:]
                    )
            psz_tile = psz.tile([_B, 4 * _UL], f32, tag="z")
            for j in range(2):
                nc.tensor.matmul(
                    psz_tile[:],
                    xt[:, j, :],
                    ker_t[:, j, :],
                    start=(j == 0),
                    stop=False,
                )
            for n in range(_KH):
                nc.tensor.matmul(
                    psz_tile[:],
                    xt[:, _KX + n, :],
                    rec_t[:, n, :],
                    start=False,
                    stop=False,
                )

            # keep PE busy through the collective window so the tail
            # matmuls run at full pstate (idle PE drops to 1.2 GHz)
            warm_ps = pstp.tile([1, 512], f32, tag="den")
            for _w in range(16):
                nc.tensor.matmul(
                    warm_ps[:],
                    ones_t[0:1, 0:1],
                    bia_t[:, 0:512],
                    start=True,
                    stop=True,
                )

            ctx_full = smallp.tile([_B, _E], mybir.dt.bfloat16)
            nc.scalar.dma_start(ctx_full[:], cc_out[:])

            # transpose ctx_full into xt[:, 2..9, :]
            identb = constp.tile([32, 32], mybir.dt.bfloat16)
            nc.vector.tensor_copy(identb[:], ident_t)
            for n in range(_KH):
                tp = pstp.tile([128, _B], mybir.dt.bfloat16, tag="tp")
                nc.tensor.transpose(
                    tp[:],
                    ctx_full[:, n * 128 : (n + 1) * 128],
                    identb[:],
                )
                nc.vector.tensor_copy(xt[:, 2 + n, :], tp[:])

            # ---------- finish z: ctx contraction + bias ----------
            for j in range(2, _KX):
                nc.tensor.matmul(
                    psz_tile[:],
                    xt[:, j, :],
                    ker_t[:, j, :],
                    start=False,
                    stop=False,
                )
            nc.tensor.matmul(
                psz_tile[:],
                _r(ones_t[0:1, 0:_B]),
                _r(bia_t[:]),
                start=False,
                stop=True,
            )

            # gates: [zi | zf | zg | zo] each [B, UL]
            si = smallp.tile([_B, _UL], f32)
            sf = smallp.tile([_B, _UL], f32)
            tg = smallp.tile([_B, _UL], f32)
            so = smallp.tile([_B, _UL], f32)
            nc.scalar.activation(si[:], psz_tile[:, 0 * _UL : 1 * _UL], AF.Sigmoid)
            nc.scalar.activation(sf[:], psz_tile[:, 1 * _UL : 2 * _UL], AF.Sigmoid)
            nc.scalar.activation(tg[:], psz_tile[:, 2 * _UL : 3 * _UL], AF.Tanh)
            nc.scalar.activation(so[:], psz_tile[:, 3 * _UL : 4 * _UL], AF.Sigmoid)
            hc = smallp.tile([_B, 2 * _UL], f32)
            hn = hc[:, 0:_UL]
            cn = hc[:, _UL : 2 * _UL]
            t1 = smallp.tile([_B, _UL], f32)
            nc.vector.tensor_tensor(out=t1[:], in0=si[:], in1=tg[:], op=ALU.mult)
            t2 = smallp.tile([_B, _UL], f32)
            nc.vector.tensor_tensor(out=t2[:], in0=sf[:], in1=c_t[:], op=ALU.mult)
            nc.vector.tensor_tensor(out=cn, in0=t1[:], in1=t2[:], op=ALU.add)
            tc_t = smallp.tile([_B, _UL], f32)
            nc.scalar.activation(tc_t[:], cn, AF.Tanh)
            nc.vector.tensor_tensor(out=hn, in0=so[:], in1=tc_t[:], op=ALU.mult)

            # one DMA: DRAM rows (g*B + b) <- SBUF partition b, half g
            nc.sync.dma_start(
                out.ap().rearrange("g b u -> b g u"),
                hc[:].rearrange("b (g u) -> b g u", g=2),
            )

    nc.compile()
    return nc


def _get_nc():
    if "nc" not in _CACHE:
        _CACHE["nc"] = _build()
    return _CACHE["nc"]


def _prepare_in_maps(
    inputs, h, c, speech_encode, encodestate, Wa_w, Wa_b, va_w, kernel, rec_kernel, bias
):
    f = np.float32
    inputs = np.ascontiguousarray(inputs, f)
    h = np.ascontiguousarray(h, f)
    c = np.ascontiguousarray(c, f)
    speech_encode = np.ascontiguousarray(speech_encode, f)
    encodestate = np.ascontiguousarray(encodestate, f)

    import ml_dtypes
    bf16 = ml_dtypes.bfloat16
    xinT = np.ascontiguousarray(inputs.T.astype(bf16))  # [DIN, B]
    hT_f = np.ascontiguousarray(h.T)  # [U, B] f32 (for hTc slicing)
    hT = np.ascontiguousarray(hT_f.astype(bf16))
    wa_ext = np.zeros((_KQ * 128, _U), f)
    wa_ext[:_U] = np.asarray(Wa_w, f)
    wa_ext[_U] = np.asarray(Wa_b, f).reshape(_U)
    wa_bf = np.ascontiguousarray(wa_ext.astype(bf16))
    hTc_ext = np.zeros((_KQ * 128, _B), f)
    hTc_ext[:_U] = hT_f
    hTc_ext[_U] = 1.0
    va = np.ascontiguousarray(np.asarray(va_w, f).reshape(_U, 1).T)  # [1, U]
    # interleaved column shards: gate-major [4, R, UL]
    ker4 = np.ascontiguousarray(kernel, f).reshape(_DIN + _E, 4, _R, _UL)
    rec4 = np.ascontiguousarray(rec_kernel, f).reshape(_U, 4, _R, _UL)
    bia4 = np.ascontiguousarray(bias, f).reshape(4, _R, _UL)

    in_maps = []
    for r in range(_R):
        in_maps.append(
            {
                "xinT": xinT,
                "hT": hT,
                "hTc": np.ascontiguousarray(
                    hTc_ext[:, r * _BL : (r + 1) * _BL].astype(bf16)
                ),
                "c_sh": np.ascontiguousarray(c[:, r * _UL : (r + 1) * _UL]),
                "enc": np.ascontiguousarray(
                    encodestate[r * _BL : (r + 1) * _BL].astype(bf16)
                ),
                "spe": np.ascontiguousarray(
                    speech_encode[r * _BL : (r + 1) * _BL].astype(bf16)
                ),
                "wa": wa_bf,
                "va": va,
                "ker": np.ascontiguousarray(
                    ker4[:, :, r, :].astype(bf16)
                ).reshape(_DIN + _E, 4 * _UL),
                "rec": np.ascontiguousarray(
                    rec4[:, :, r, :].astype(bf16)
                ).reshape(_U, 4 * _UL),
                "bia": np.ascontiguousarray(bia4[:, r, :]).reshape(1, 4 * _UL),
            }
        )
    return in_maps


def _postprocess(results):
    f = np.float32
    h_new = np.empty((_B, _U), f)
    c_new = np.empty((_B, _U), f)
    for r in range(_R):
        o = results[r]["out"]
        h_new[:, r * _UL : (r + 1) * _UL] = o[0]
        c_new[:, r * _UL : (r + 1) * _UL] = o[1]
    return np.stack([h_new, h_new, c_new], axis=0)


def kernel(
    inputs,
    h,
    c,
    speech_encode,
    encodestate,
    Wa_w,
    Wa_b,
    va_w,
    va_b,
    kernel,
    rec_kernel,
    bias,
):
    from concourse import bass_utils

    in_maps = _prepare_in_maps(
        inputs, h, c, speech_encode, encodestate, Wa_w, Wa_b, va_w,
        kernel, rec_kernel, bias,
    )
    nc = _get_nc()
    res = bass_utils.run_bass_kernel_spmd(nc, in_maps, core_ids=list(range(_R)))
    return _postprocess(res.results)
